# revision 8
# baseline (speedup 1.0000x reference)
"""Trainium2 Bass kernel for nn_Model2 (5x 4D conv + 2 FC), data-parallel over 8 cores.

V2 redesign vs baseline:
  - L1 (ci=1, the PE-fill disaster of the single-axis Toeplitz: K=18,M=45) now
    uses a DOUBLE-window Toeplitz: partitions hold an (i3,i4) window, lhsT
    contracts (d3,d4) at once, accumulation is only over (d1,d2): 16 steps
    instead of 64, with K up to 88 / M up to 120.  ~10x fewer PE cycles.
  - act2 layout is pos4-major (o4*3+ci) so L1 psum slices evacuate to
    partition-contiguous ranges.
  - L2 runs as TWO concurrent streams via PE row-tiling (K=45<=64): batch
    half A at partitions 0-44, half B at 64-108, interleaved matmuls run
    concurrently in disjoint row-groups of the PE array -> ~2x L2.
  - Batch pipelined in 4 groups of 8 elements to fit SBUF; all activations
    stay on-chip.
  - Evacuations (bias+ReLU) split across Scalar and Vector engines.
L3..L5 + FC head keep the baseline single-axis Toeplitz scheme.
"""
import numpy as np
import ml_dtypes
from contextlib import ExitStack
from itertools import product

# ---------------- hardcoded problem config ----------------
B = 256
N_CORES = 8
B_SH = B // N_CORES          # 32 per core
G_N = 4                      # batch groups per core
G_SZ = B_SH // G_N           # 8 elems per group (4 at base 0, 4 at base 64)
CONV_CFG = [(1, 3, 4), (3, 9, 4), (9, 12, 4), (12, 15, 4), (15, 15, 3)]
S_IN = 18
FLAT = 15 * 4 ** 4
FC1_N = 100

# L1 double-window config.  Engine APs must start at 32-aligned partitions,
# so each j3 (o3-within-chunk) block of the psum M dim is padded to a 32-row
# stride, and act2's two o4-chunk blocks sit at partition 0 / 32 (+64 for
# the B batch-half).  W2's Toeplitz rows carry matching zero pad rows.
A3_CHUNKS = [(0, 4), (4, 4), (8, 4), (12, 3)]   # (o3 start, c3)
C4_CHUNKS = [(0, 8), (8, 7)]                    # (o4 start, c4)
K1MAX = 7 * 11                                  # max (c3+3)*(c4+3) = 77
K2P = 53                                        # padded L2 contraction rows
K2X = 117                                       # d3-folded rows: 53 + gap + 53

_BF16 = ml_dtypes.bfloat16


class _LayerCfg:
    def __init__(self, ci, co, k, sin):
        self.ci, self.co, self.k, self.sin = ci, co, k, sin
        self.sout = sin - k + 1
        self.K = ci * sin
        self.M = co * self.sout
        self.n_acc = k ** 3


def _layers():
    ls, s = [], S_IN
    for ci, co, k in CONV_CFG:
        L = _LayerCfg(ci, co, k, s)
        ls.append(L)
        s = L.sout
    return ls


def _a2row(i4, ci):
    """act2 partition row for (i4, ci): o4-chunk blocks at 0 and 32."""
    if i4 < 8:
        return i4 * 3 + ci
    return 32 + (i4 - 8) * 3 + ci


def _build_w1(w1, c3, c4):
    """W[(t3,t4), (d1,d2), 32*j3 + (j4*3+co)] = w1[co,0,d1,d2,t3-j3,t4-j4]."""
    k = 4
    w3, w4 = c3 + k - 1, c4 + k - 1
    W = np.zeros((w3 * w4, k * k, 32 * c3), np.float32)
    wv = np.asarray(w1, np.float32)
    for d1, d2 in product(range(k), repeat=2):
        for j3 in range(c3):
            for j4 in range(c4):
                for d3 in range(k):
                    for d4 in range(k):
                        W[(j3 + d3) * w4 + (j4 + d4), d1 * k + d2,
                          32 * j3 + j4 * 3 + np.arange(3)] = wv[:, 0, d1, d2, d3, d4]
    return W


def _build_w2x(w2):
    """d3-folded L2 weights [K2X=117, 32, 108].

    Partition rows: block0 (0-52) = _a2row(i4,ci) for d3 = d3p; block1
    (64-116) = same rows for d3 = d3p+1 (fed by the i3-shifted act2 copy).
    Rows 24-31, 53-63, 88-95 are zero pads.  Steps: (d1,d2,d3p) with
    d3p in {0,2} -> 32 accumulation steps.  Cols (co*12+o4)."""
    k, ci_n, co_n, sin, sout = 4, 3, 9, 15, 12
    W = np.zeros((K2X, 32, co_n * sout), np.float32)
    wv = np.asarray(w2, np.float32)
    for d1, d2 in product(range(k), repeat=2):
        for pi, d3p in enumerate((0, 2)):
            st = (d1 * k + d2) * 2 + pi
            for d3o, base in ((0, 0), (1, 64)):
                d3 = d3p + d3o
                for d4 in range(k):
                    for o4 in range(sout):
                        for ci in range(ci_n):
                            W[base + _a2row(o4 + d4, ci), st,
                              np.arange(co_n) * sout + o4] = wv[:, ci, d1, d2, d3, d4]
    return W


def _toeplitz_weights(w, L):
    """baseline ch-major: rows (ci*sin+i4), cols (co*sout+o4)."""
    k, ci_n, co_n, sin, sout = L.k, L.ci, L.co, L.sin, L.sout
    Wt = np.zeros((L.K, L.n_acc, L.M), np.float32)
    wv = np.asarray(w, np.float32)
    ci_idx = np.arange(ci_n)
    co_idx = np.arange(co_n)
    for d1, d2, d3 in product(range(k), repeat=3):
        di = (d1 * k + d2) * k + d3
        for d4 in range(k):
            for o4 in range(sout):
                i4 = o4 + d4
                blk = wv[:, :, d1, d2, d3, d4]
                Wt[(ci_idx * sin + i4)[:, None], di,
                   (co_idx * sout + o4)[None, :]] = blk.T
    return Wt


def _host_prep(inputs):
    x = np.asarray(inputs["x"], np.float32)
    Ls = _layers()
    consts = {}
    # L1 double-window weights: one variant per (c3, c4); padded K/M layout
    for c3 in (4, 3):
        for c4 in (8, 7):
            W = _build_w1(inputs["w1"], c3, c4)
            Wp = np.zeros((K1MAX, 16, 128), np.float32)
            Wp[:W.shape[0], :, :W.shape[2]] = W
            consts[f"cw1_{c3}_{c4}"] = np.ascontiguousarray(Wp).astype(_BF16)
    # bias row pattern: value b1[(r%32)%3] (pad rows unused)
    b1 = np.asarray(inputs["b1"], np.float32)
    consts["cb1t"] = np.ascontiguousarray(
        b1[(np.arange(128) % 32) % 3].reshape(128, 1))
    # L2 pos4-major rows, d3-folded
    consts["cw2"] = np.ascontiguousarray(_build_w2x(inputs["w2"])).astype(_BF16)
    consts["cb2"] = np.ascontiguousarray(
        np.repeat(np.asarray(inputs["b2"], np.float32), 12).reshape(108, 1))
    # L3..L5 baseline
    for i in (3, 4, 5):
        L = Ls[i - 1]
        Wt = _toeplitz_weights(inputs[f"w{i}"], L)
        consts[f"cw{i}"] = np.ascontiguousarray(Wt).astype(_BF16)
        consts[f"cb{i}"] = np.ascontiguousarray(
            np.repeat(np.asarray(inputs[f"b{i}"], np.float32), L.sout).reshape(L.M, 1))
    f1 = np.asarray(inputs["fc1_w"], np.float32).reshape(15, 4, 4, 4, 4, FC1_N)
    f1 = f1.transpose(0, 4, 1, 2, 3, 5).reshape(60, 64, FC1_N)
    consts["cfc1w"] = np.ascontiguousarray(f1).astype(_BF16)
    consts["cfc1b"] = np.ascontiguousarray(
        np.asarray(inputs["fc1_b"], np.float32).reshape(FC1_N, 1))
    consts["cfc2w"] = np.ascontiguousarray(
        np.asarray(inputs["fc2_w"], np.float32)).astype(_BF16)
    fc2b = float(np.asarray(inputs["fc2_b"], np.float32).reshape(-1)[0])

    # x per core, windowed for L1: xw [16 pairs, 4 a3, 2 a4, K1MAX, 648]
    # rows (t3,t4) over window (i3 in [o3s, o3s+c3+3), i4 win), free (b, i1, i2)
    shards = []
    for c in range(N_CORES):
        xs = x[c * B_SH:(c + 1) * B_SH, 0]             # [32,18,18,18,18] (i1,i2,i3,i4)
        xw = np.zeros((16, 4, 2, K1MAX, 648), np.float32)
        for p in range(16):
            for a3, (o3s, c3) in enumerate(A3_CHUNKS):
                w3 = c3 + 3
                for a4, (o4s, c4) in enumerate(C4_CHUNKS):
                    w4 = c4 + 3
                    w = xs[2 * p:2 * p + 2, :, :, o3s:o3s + w3, o4s:o4s + w4]
                    xw[p, a3, a4, :w3 * w4] = \
                        w.transpose(3, 4, 0, 1, 2).reshape(w3 * w4, 648)
        shards.append(np.ascontiguousarray(xw).astype(_BF16))
    return Ls, consts, fc2b, shards


def _in_maps(shards):
    return [{"xw": s} for s in shards]


def _build_module(Ls, consts, fc2b):
    import concourse.bass as bass
    import concourse.tile as tile
    import concourse.mybir as mybir

    dt = mybir.dt
    AF = mybir.ActivationFunctionType
    ALU = mybir.AluOpType
    nc = bass.Bass("TRN2", target_bir_lowering=False, debug=False,
                   enable_asserts=False, num_devices=N_CORES)

    xw_d = nc.dram_tensor("xw", [16, 4, 2, K1MAX, 648], dt.bfloat16,
                          kind="ExternalInput").ap()
    out_d = nc.dram_tensor("out", [1, B_SH], dt.float32, kind="ExternalOutput").ap()
    cd = {k: nc.inline_tensor(v, name=k) for k, v in consts.items()}

    with tile.TileContext(nc) as tc:
        with ExitStack() as ctx:
            wpool = ctx.enter_context(tc.tile_pool(name="wts", bufs=1))
            xpool = ctx.enter_context(tc.tile_pool(name="xin", bufs=3))
            a3pool = ctx.enter_context(tc.tile_pool(name="act3", bufs=1))
            a4pool = ctx.enter_context(tc.tile_pool(name="act4", bufs=2))
            a5pool = ctx.enter_context(tc.tile_pool(name="act5", bufs=2))
            hpool = ctx.enter_context(tc.tile_pool(name="hbuf", bufs=1))
            opool = ctx.enter_context(tc.tile_pool(name="outb", bufs=1))
            pspool = ctx.enter_context(tc.tile_pool(name="ps", bufs=5, space="PSUM"))
            fcps = ctx.enter_context(tc.tile_pool(name="fcps", bufs=1, space="PSUM"))

            # ---- constants ----
            w1t = {}
            for c3 in (4, 3):
                for c4 in (8, 7):
                    t = wpool.tile([K1MAX, 16, 128], dt.bfloat16, tag=f"w1_{c3}_{c4}")
                    nc.sync.dma_start(t[:], cd[f"cw1_{c3}_{c4}"].ap())
                    w1t[(c3, c4)] = t
            b1t = wpool.tile([128, 1], dt.float32, tag="b1t")
            nc.sync.dma_start(b1t[:], cd["cb1t"].ap())
            # d3-folded w2: rows 0-52 (d3=d3p) + 64-116 (d3=d3p+1), 32 steps
            w2 = wpool.tile([K2X, 32, 108], dt.bfloat16, tag="w2")
            nc.sync.dma_start(w2[:], cd["cw2"].ap())
            b2 = wpool.tile([108, 1], dt.float32, tag="b2")
            nc.sync.dma_start(b2[:], cd["cb2"].ap())
            w_sb, b_sb = {}, {}
            for i in (3, 4, 5):
                L = Ls[i - 1]
                wt = wpool.tile([L.K, L.n_acc, L.M], dt.bfloat16, tag=f"w{i}")
                nc.sync.dma_start(wt[:], cd[f"cw{i}"].ap())
                w_sb[i] = wt
                bt = wpool.tile([L.M, 1], dt.float32, tag=f"b{i}")
                nc.sync.dma_start(bt[:], cd[f"cb{i}"].ap())
                b_sb[i] = bt
            fc1w = wpool.tile([60, 64, FC1_N], dt.bfloat16, tag="fc1w")
            nc.sync.dma_start(fc1w[:], cd["cfc1w"].ap())
            fc1b = wpool.tile([FC1_N, 1], dt.float32, tag="fc1b")
            nc.sync.dma_start(fc1b[:], cd["cfc1b"].ap())
            fc2w = wpool.tile([FC1_N, 1], dt.bfloat16, tag="fc2w")
            nc.sync.dma_start(fc2w[:], cd["cfc2w"].ap())
            zt = wpool.tile([128, 1], dt.float32, tag="zt")
            nc.vector.memset(zt[:], 0.0)

            h_all = hpool.tile([60, B_SH, 4, 4, 4], dt.bfloat16)
            # act2 free layout is (slot, i3, i1, i2); rows 0-52 = _a2row(i4,ci),
            # rows 64-116 = i3-shifted (+1) copy for the L2 d3-fold.  3 tiles
            # round-robin over (group, half).  Full memset once so pad rows
            # (24-31, 53-63, 88-95) are 0, not uninitialized (bf16 NaN x 0 =
            # NaN would poison psum).
            act2_bufs = []
            for i in range(3):
                t = hpool.tile([128, 4, 15, 15, 15], dt.bfloat16, tag=f"act2_{i}")
                nc.vector.memset(t[:].rearrange("p a b c d -> p (a b c d)"), 0.0)
                act2_bufs.append(t)

            L3, L4, L5 = Ls[2], Ls[3], Ls[4]

            def evac(engine, dst, ps_ap, bias_ap, nparts, zbase=0):
                """dst = relu(ps + bias). engine: 's' scalar | 'v' vector.
                DVE path (InstTensorScalarPtr): APs must canonicalize to <=3
                dims, and both SBUF inputs (bias, zeros) need the same base
                partition - callers pass zbase = bias base."""
                if engine == "s":
                    nc.scalar.activation(dst, ps_ap, AF.Relu, bias=bias_ap)
                else:
                    sh = list(dst.shape)
                    assert len(sh) <= 3, f"DVE evac needs <=3d dst, got {sh}"
                    nc.vector.scalar_tensor_tensor(
                        dst, ps_ap, bias_ap,
                        zt[zbase:zbase + nparts, :].to_broadcast(tuple(sh)),
                        op0=ALU.add, op1=ALU.max)

            for g in range(G_N):
                # =================== L1 ===================
                # group g covers elems 8g..8g+7 -> pairs 4g..4g+3.
                # pairs 4g,4g+1 -> half-A tile slots 0-3; 4g+2,4g+3 -> half-B.
                tA = act2_bufs[(2 * g) % 3]
                tB = act2_bufs[(2 * g + 1) % 3]
                for ph in range(4):               # pair within group
                    pr = 4 * g + ph
                    td = tA if ph < 2 else tB
                    slot0 = (ph % 2) * 2
                    for a3, (o3s, c3) in enumerate(A3_CHUNKS):
                        w3 = c3 + 3
                        for a4, (o4s, c4) in enumerate(C4_CHUNKS):
                            w4 = c4 + 3
                            K1 = w3 * w4
                            w1 = w1t[(c3, c4)]
                            xt = xpool.tile([K1MAX, 2, 324], dt.bfloat16, tag="xt")
                            nc.sync.dma_start(
                                xt[0:K1], xw_d[pr, a3, a4, 0:K1].rearrange(
                                    "p (b f) -> p b f", b=2))
                            ps = pspool.tile([128, 2, 15, 15], dt.float32, tag="ps")
                            for d1 in range(4):
                                for d2 in range(4):
                                    rhs = xt[0:K1].rearrange(
                                        "p b (i j) -> p b i j", i=18)[
                                        :, :, d1:d1 + 15, d2:d2 + 15]
                                    nc.tensor.matmul(
                                        ps[0:32 * c3], w1[0:K1, d1 * 4 + d2, 0:32 * c3],
                                        rhs,
                                        start=(d1 == 0 and d2 == 0),
                                        stop=(d1 == 3 and d2 == 3))
                            for j3 in range(c3):
                                sl = ps[32 * j3:32 * j3 + 3 * c4, :, :, :]
                                dst = td[32 * a4:32 * a4 + 3 * c4,
                                         slot0:slot0 + 2, o3s + j3, :, :]
                                if j3 % 2 == 0:
                                    evac("s", dst, sl,
                                         b1t[32 * j3:32 * j3 + 3 * c4, :], 3 * c4)
                                else:
                                    evac("v", dst.rearrange("p b i j -> p b (i j)"),
                                         sl.rearrange("p b i j -> p b (i j)"),
                                         b1t[32 * j3:32 * j3 + 3 * c4, :], 3 * c4,
                                         zbase=32 * j3)

                # i3-shifted duplicate (rows 64-116) feeding the w2 d3-fold
                for th in (tA, tB):
                    nc.gpsimd.tensor_scalar_add(
                        th[64:K2X, :, 0:14, :, :].rearrange(
                            "p s t a b -> p s t (a b)"),
                        th[0:53, :, 1:15, :, :].rearrange(
                            "p s t a b -> p s t (a b)"),
                        0.0)

                # =================== L2 (d3-folded, 32 steps) ===================
                act3 = a3pool.tile([108, G_SZ, 12, 12, 12], dt.bfloat16, tag="act3")
                for half, th in ((0, tA), (1, tB)):
                    for sl in range(4):           # slot within half = batch elem
                        for o1a in range(0, 12, 3):
                            ps = pspool.tile([108, 3, 12, 12], dt.float32, tag="ps")
                            di = 0
                            for d1, d2 in product(range(4), repeat=2):
                                for d3p in (0, 2):
                                    rhs = th[0:K2X, sl, d3p:d3p + 12,
                                             d1 + o1a:d1 + o1a + 3,
                                             d2:d2 + 12].rearrange(
                                                 "p c a b -> p a b c")
                                    nc.tensor.matmul(
                                        ps[:], w2[0:K2X, di, :], rhs,
                                        start=(di == 0), stop=(di == 31))
                                    di += 1
                            hs = half * 4 + sl
                            if (sl + o1a) % 2 == 0:
                                evac("s", act3[:, hs, o1a:o1a + 3, :, :], ps[:],
                                     b2[:], 108)
                            else:
                                evac("v",
                                     act3[:, hs, o1a:o1a + 3, :, :].rearrange(
                                         "p a b c -> p (a b c)"),
                                     ps[:].rearrange("p a b c -> p (a b c)"),
                                     b2[:], 108)

                # =================== L3 ===================
                act4 = a4pool.tile([108, G_SZ, 9, 9, 9], dt.bfloat16, tag="act4")
                for sl in range(G_SZ):
                    for ci_, o1a in enumerate((0, 6)):
                        no1 = 6 if o1a == 0 else 3
                        ps = pspool.tile([108, 6, 9, 9], dt.float32, tag="ps")
                        di = 0
                        for d1, d2, d3 in product(range(4), repeat=3):
                            rhs = act3[:, sl, d1 + o1a:d1 + o1a + no1,
                                       d2:d2 + 9, d3:d3 + 9]
                            nc.tensor.matmul(ps[0:108, 0:no1, :, :],
                                             w_sb[3][:, di, :], rhs,
                                             start=(di == 0), stop=(di == 63))
                            di += 1
                        if sl % 2 == 0:
                            evac("s", act4[:, sl, o1a:o1a + no1, :, :],
                                 ps[0:108, 0:no1, :, :], b_sb[3][:], 108)
                        else:
                            evac("v",
                                 act4[:, sl, o1a:o1a + no1, :, :].rearrange(
                                     "p a b c -> p (a b c)"),
                                 ps[0:108, 0:no1, :, :].rearrange(
                                     "p a b c -> p (a b c)"),
                                 b_sb[3][:], 108)

                # =================== L4 ===================
                act5 = a5pool.tile([90, G_SZ, 6, 6, 6], dt.bfloat16, tag="act5")
                for sl in range(G_SZ):
                    ps = pspool.tile([90, 6, 6, 6], dt.float32, tag="ps")
                    di = 0
                    for d1, d2, d3 in product(range(4), repeat=3):
                        rhs = act4[:, sl, d1:d1 + 6, d2:d2 + 6, d3:d3 + 6]
                        nc.tensor.matmul(ps[:], w_sb[4][:, di, :], rhs,
                                         start=(di == 0), stop=(di == 63))
                        di += 1
                    if sl % 2 == 0:
                        evac("s", act5[:, sl, :, :, :], ps[:], b_sb[4][:], 90)
                    else:
                        evac("v",
                             act5[:, sl, :, :, :].rearrange("p a b c -> p (a b c)"),
                             ps[:].rearrange("p a b c -> p (a b c)"),
                             b_sb[4][:], 90)

                # =================== L5 ===================
                for sl in range(G_SZ):
                    ps = pspool.tile([60, 4, 4, 4], dt.float32, tag="ps")
                    di = 0
                    for d1, d2, d3 in product(range(3), repeat=3):
                        rhs = act5[:, sl, d1:d1 + 4, d2:d2 + 4, d3:d3 + 4]
                        nc.tensor.matmul(ps[:], w_sb[5][:, di, :], rhs,
                                         start=(di == 0), stop=(di == 26))
                        di += 1
                    evac("s", h_all[:, g * G_SZ + sl, :, :, :], ps[:],
                         b_sb[5][:], 60)

            # ---- FC head ----
            ps1 = fcps.tile([FC1_N, B_SH], dt.float32, tag="psfc")
            for f in range(64):
                x0, y0, z0 = f // 16, (f // 4) % 4, f % 4
                nc.tensor.matmul(ps1[:], fc1w[:, f, :], h_all[:, :, x0, y0, z0],
                                 start=(f == 0), stop=(f == 63))
            r1 = opool.tile([FC1_N, B_SH], dt.bfloat16, tag="r1")
            nc.scalar.activation(r1[:], ps1[:], AF.Relu, bias=fc1b[:])
            ps2 = fcps.tile([1, B_SH], dt.float32, tag="psfc2")
            nc.tensor.matmul(ps2[:], fc2w[:], r1[:], start=True, stop=True)
            ob = opool.tile([1, B_SH], dt.float32, tag="ob")
            nc.scalar.activation(ob[:], ps2[:], AF.Sigmoid, bias=fc2b)
            nc.sync.dma_start(out_d, ob[:])

    _split_excess_waits(nc)
    return nc


def _split_excess_waits(nc, max_waits=1):
    """Hoist excess sem-waits onto injected wait-only carriers (toolchain
    accepts very few waits per instruction)."""
    import concourse.mybir as mybir
    f = nc.m.functions[0]
    ctr = 0
    for blk in f.blocks:
        il = blk.instructions
        i = 0
        while i < len(il):
            inst = il[i]
            si = inst.sync_info
            ty = type(inst).__name__
            lim = 2 if ty == "InstEventSemaphore" else max_waits
            if si is not None and si.on_wait and len(si.on_wait) > lim:
                waits = list(si.on_wait)
                si.on_wait = waits[:lim]
                for w in waits[lim:]:
                    ev = mybir.InstEventSemaphore(name=f"wsplit_{ctr}", ins=[], outs=[])
                    ctr += 1
                    ev.engine = inst.engine
                    ev.sync_info = mybir.SyncInfo(on_wait=[w], on_update=[])
                    il.insert(i, ev)
                    i += 1
            i += 1
    return ctr


def kernel(**inputs) -> np.ndarray:
    import concourse.bass_utils as bass_utils
    Ls, consts, fc2b, shards = _host_prep(inputs)
    nc = _build_module(Ls, consts, fc2b)
    in_maps = _in_maps(shards)
    r = bass_utils.run_bass_kernel_spmd(nc, in_maps, core_ids=list(range(N_CORES)))
    outs = [r.results[c]["out"].reshape(B_SH, 1) for c in range(N_CORES)]
    return np.concatenate(outs, axis=0).astype(np.float32)


if __name__ == "__main__":
    rng = np.random.default_rng(0)
    ins = {"x": rng.normal(size=(B, 1, 18, 18, 18, 18)).astype(np.float32)}
    for i, (ci, co, k) in enumerate(CONV_CFG, start=1):
        ins[f"w{i}"] = (rng.normal(size=(co, ci, k, k, k, k)) / np.sqrt(ci * k ** 4)).astype(np.float32)
        ins[f"b{i}"] = np.zeros((co,), np.float32)
    ins["fc1_w"] = (rng.normal(size=(FLAT, FC1_N)) / np.sqrt(FLAT)).astype(np.float32)
    ins["fc1_b"] = np.zeros((FC1_N,), np.float32)
    ins["fc2_w"] = (rng.normal(size=(FC1_N, 1)) / 10.0).astype(np.float32)
    ins["fc2_b"] = np.zeros((1,), np.float32)
    out = kernel(**ins)
    print("out", out.shape, out[:4, 0])

